# revision 21
# baseline (speedup 1.0000x reference)
"""Trainium2 Bass kernel for capsule attention-routing.

Reference computation (per pixel; 4096 independent problems of shape
[I=32 in-caps, N=32 out-caps, J=16 caps-dim]):
    v[n,j]   = sum_i u[i,n,j]
    cp[i,n]  = sum_j u[i,n,j] * v[n,j] / 4
    c[i,n]   = softmax_n(cp)[i,n] + b[i,n]
    s[n,j]   = sum_i u[i,n,j] * c[i,n]
    out[n,j] = (1 - exp(-|s|_j)) * s[n,j] / |s|_j

Sharding: data-parallel over (batch, h-half): 8 cores x 512 pixels.

Per-core strategy (dual layout, u streamed twice from HBM as fp16):
  L1 (j-major): partitions (j*8+il), free (ib, n, p64)  [il=i%8, i=ib*8+il]
     - v-pass: PE contracts il (+PSUM accum over ib), broadcast over rows
     - c-mult: DVE 2x-mode fp16 w = u1 * v
     - c-red : PE contracts j via banded 0.25-delta weights ->
               cp[(g*8+il) parts, (ib,n,p4)]  [g = pixel>>2]
  softmax over n on the small cp tile (Act exp f32, Pool z-reduce,
  DVE reciprocal+mult -> c_sb fp16)
  L2 (i-major): partitions (g*8+il), free (ib, n, j, p4)
     - s-mult: DVE 2x-mode m2 = u2 * broadcast_j(c_sb)  (no PE broadcast,
               no PSUM->SBUF copies: c broadcasts via a stride-0 free dim)
     - s-red : PE contracts il within g strips (+accum over ib) ->
               s[(g,x8-replicated) parts, (n8,j,p4)] in 4 nq PSUM banks
  squash: Act square, Pool j-reduce, r = exp(.5*ln(n2)) / rn = exp(-.5*ln n2)
  (single act table: no LoadActFuncSet flips), Pool final multiply.
Softmax runs without max-subtraction (|cp| <~ 45 is safe in fp32 exp).
EPS=1e-20 is negligible: 1-1/(exp(r)+eps) == 1-exp(-r), 1/(r+eps) == 1/r.
"""

import numpy as np
from contextlib import ExitStack

import concourse.bass as bass
import concourse.bacc as bacc
import concourse.tile as tile
import concourse.mybir as mybir
from concourse.bass_utils import run_bass_kernel_spmd

dt = mybir.dt
AF = mybir.ActivationFunctionType
OP = mybir.AluOpType

B, I, N, J, H, W = 4, 32, 32, 16, 32, 32
HW = H * W
NCORES = 8
PIX = B * HW // NCORES      # 512 pixels per core
BLK = 64                    # pixels per block
NBLK = PIX // BLK           # 8
NG = 16                     # pixel groups of 4 per block (g = pixel>>2)
P4 = 4
SCALE = 0.25                # 1/sqrt(16)

f32, bf16, f16 = dt.float32, dt.bfloat16, dt.float16


def _build_weight_arrays():
    il_of = np.arange(128) % 8          # L1 partition -> il is p%8? no: p=(j,il)
    # L1 partitions: p = j*8 + il  -> j = p//8, il = p%8
    j_of = np.arange(128) // 8
    il1 = np.arange(128) % 8

    # v-pass: out[(j2,il2)] = sum_il u[(j,il)] for j==j2 (broadcast over il2)
    wv = np.zeros((128, 128), np.float32)
    for p_in in range(128):
        for p_out in range(128):
            if j_of[p_in] == j_of[p_out]:
                wv[p_in, p_out] = 1.0

    # c-red band: window at offset off(g)=2*(120 - g*8) bytes gives the
    # [128,128] weight mapping (j,il) -> out partition (g*8+il), scaled 0.25.
    # band[(j,il), c] = 0.25 iff c == 120 + il
    wc_band = np.zeros((128, 248), np.float32)
    for p_in in range(128):
        wc_band[p_in, 120 + il1[p_in]] = SCALE

    # s-red band: window at offset off(jq)=2*(6 - jq*2) bytes maps L2
    # partitions (g,il) -> out partition (g*8 + jq*2 + r), r=0,1 replicas.
    # band[(g,il), c] = 1 iff c in (g*8+6, g*8+7)
    ws_band = np.zeros((128, 134), np.float32)
    g_of = np.arange(128) // 8
    for p_in in range(128):
        ws_band[p_in, g_of[p_in] * 8 + 6] = 1.0
        ws_band[p_in, g_of[p_in] * 8 + 7] = 1.0

    # n2: contract the 8 rows of each g strip (each real value appears
    # twice via the r2 replicas -> 0.5)
    wn = np.zeros((128, 128), np.float32)
    for p_in in range(128):
        for p_out in range(128):
            if p_in // 8 == p_out // 8:
                wn[p_in, p_out] = 0.5

    return {"wv": wv, "wc_band": wc_band, "ws_band": ws_band, "wn": wn}


def _b_tile_array(b_np):
    # bt[(g*8+il), (ib, n, p4)] = b[ib*8+il, n]
    bt = np.zeros((128, 4 * N * P4), np.float32)
    bsl = np.asarray(b_np).reshape(I, N)
    for g in range(NG):
        for il in range(8):
            row = g * 8 + il
            for ib in range(4):
                for n in range(N):
                    bt[row, (ib * N + n) * P4 : (ib * N + n + 1) * P4] = bsl[
                        ib * 8 + il, n
                    ]
    return bt


def _emit(ctx: ExitStack, tc: tile.TileContext, aps: dict, with_b: bool):
    nc = tc.nc
    u1_d, u2_d, o_d = aps["u1"], aps["u2"], aps["out"]

    # constants
    pconst = ctx.enter_context(tc.tile_pool(name="const", bufs=1))
    wv_t = pconst.tile([128, 128], f16, tag="wv")
    wcb_t = pconst.tile([128, 248], f16, tag="wcb")
    ws_t = pconst.tile([128, 134], f16, tag="ws")
    wn_t = pconst.tile([128, 128], f16, tag="wn")
    nc.sync.dma_start(wv_t[:], aps["wv"])
    nc.sync.dma_start(wcb_t[:], aps["wc_band"])
    nc.sync.dma_start(ws_t[:], aps["ws_band"])
    nc.sync.dma_start(wn_t[:], aps["wn"])
    bt_t = None
    if with_b:
        bt_t = pconst.tile([128, 4 * N * P4], f32, tag="bt")
        nc.sync.dma_start(bt_t[:], aps["bt"])

    # pools
    pu1 = ctx.enter_context(tc.tile_pool(name="u1", bufs=2))
    pu2 = ctx.enter_context(tc.tile_pool(name="u2", bufs=2))
    pw1 = ctx.enter_context(tc.tile_pool(name="w1", bufs=2))
    pm2 = ctx.enter_context(tc.tile_pool(name="m2", bufs=3))
    pvsb = ctx.enter_context(tc.tile_pool(name="vsb", bufs=2))
    pce = ctx.enter_context(tc.tile_pool(name="ce", bufs=2))
    pcsb = ctx.enter_context(tc.tile_pool(name="csb", bufs=2))
    psq = ctx.enter_context(tc.tile_pool(name="sq", bufs=2))
    pout = ctx.enter_context(tc.tile_pool(name="out", bufs=2))

    pvps = ctx.enter_context(tc.tile_pool(name="vps", bufs=2, space="PSUM"))
    pcps = ctx.enter_context(tc.tile_pool(name="cps", bufs=2, space="PSUM"))
    psps = ctx.enter_context(tc.tile_pool(name="sps", bufs=3, space="PSUM"))
    pnps = ctx.enter_context(tc.tile_pool(name="nps", bufs=1, space="PSUM"))

    def load(blk):
        u1 = pu1.tile([128, 4 * N * BLK], f16, tag="u1")   # [(j,il),(ib,n,p64)]
        nc.sync.dma_start(u1[:], u1_d[blk])
        u2 = pu2.tile([128, 4 * N * J * P4], f16, tag="u2")  # [(g,il),(ib,n,j,p4)]
        nc.sync.dma_start(u2[:], u2_d[blk])
        return u1, u2

    tiles = {0: load(0)}
    for blk in range(NBLK):
        # prefetch next block's loads ahead of this block's store in SP order
        if blk + 1 < NBLK:
            tiles[blk + 1] = load(blk + 1)
        u1, u2 = tiles.pop(blk)

        u1_v = u1[:].rearrange("P (ib n p) -> P ib n p", ib=4, p=BLK)

        # ---- v-pass (PE): v[(j,il-bcast),(n,p64)] = sum_i u1 ----
        v_sb = pvsb.tile([128, N * BLK], f16, tag="vsb")
        v_sb_v = v_sb[:].rearrange("P (n p) -> P n p", p=BLK)
        for st in range(4):
            v_ps = pvps.tile([128, 512], f32, tag="vps")
            v_ps_v = v_ps[:].rearrange("P (n p) -> P n p", p=16)
            for ib in range(4):
                nc.tensor.matmul(
                    v_ps_v,
                    wv_t[:],
                    u1_v[:, ib, :, st * 16 : (st + 1) * 16],
                    start=(ib == 0),
                    stop=(ib == 3),
                )
            nc.gpsimd.tensor_scalar(
                v_sb_v[:, :, st * 16 : (st + 1) * 16], v_ps_v, 0.0, None,
                op0=OP.add,
            )

        # ---- c-mult (DVE 2x): w1 = u1 * v ----
        w1 = pw1.tile([128, 4 * N * BLK], f16, tag="w1")
        w1_v = w1[:].rearrange("P (ib n p) -> P ib n p", ib=4, p=BLK)
        for ib in range(4):
            nc.vector.tensor_tensor(
                w1_v[:, ib], u1_v[:, ib], v_sb_v, op=OP.mult
            )

        # ---- c-red (PE): cp[(g,il), (ib,n,p4)] = 0.25*sum_j w1 ----
        cp = pcps.tile([128, 4 * N * P4], f32, tag="cp")
        cp_v = cp[:].rearrange("P (ib n p) -> P ib n p", ib=4, p=P4)
        for g in range(NG):
            off = 120 - g * 8
            nc.tensor.matmul(
                cp_v,
                wcb_t[:, off : off + 128],
                w1_v[:, :, :, g * P4 : (g + 1) * P4],
                start=(g == 0),
                stop=(g == NG - 1),
                skip_group_check=True,
            )

        # ---- softmax over n (no max-subtraction) ----
        c_e = pce.tile([128, 4 * N * P4], f32, tag="ce")
        nc.scalar.activation(c_e[:], cp[:], AF.Exp)
        c_e_v = c_e[:].rearrange("P (ib n p) -> P ib n p", ib=4, p=P4)
        z = pcsb.tile([128, 4 * P4], f32, tag="z")
        nc.vector.tensor_reduce(
            z[:].rearrange("P (ib p) -> P ib p", ib=4),
            c_e[:].rearrange("P (ib n p) -> P ib p n", ib=4, p=P4),
            axis=mybir.AxisListType.X,
            op=OP.add,
        )
        rz = pcsb.tile([128, 4 * P4], f32, tag="rz")
        nc.vector.reciprocal(rz[:], z[:])
        rz_b = (
            rz[:]
            .rearrange("P (ib o p) -> P ib o p", ib=4, o=1)
            .broadcast_to([128, 4, N, P4])
        )
        c_sb = pcsb.tile([128, 4 * N * P4], f16, tag="csb")
        c_sb_v = c_sb[:].rearrange("P (ib n p) -> P ib n p", ib=4, p=P4)
        if with_b:
            c_f = pcsb.tile([128, 4 * N * P4], f32, tag="cf")
            nc.gpsimd.tensor_tensor(
                c_f[:].rearrange("P (ib n p) -> P ib n p", ib=4, p=P4),
                c_e_v,
                rz_b,
                op=OP.mult,
            )
            nc.gpsimd.tensor_tensor(c_sb[:], c_f[:], bt_t[:], op=OP.add)
        else:
            nc.gpsimd.tensor_tensor(c_sb_v, c_e_v, rz_b, op=OP.mult)

        # ---- s-phase (L2): m2 = u2 * bcast_j(c_sb); s-red contracts i ----
        # s_all[(g, jq, r2) parts, (nq, m8, jl4, p4)]; j = jq*4 + jl
        u2_v = u2[:].rearrange("P (ib n j p) -> P ib n j p", ib=4, n=N, p=P4)
        s_all = psps.tile([128, 4 * 8 * 4 * P4], f32, tag="sall")
        s_all_v = s_all[:].rearrange(
            "P (q m jl p) -> P q m jl p", q=4, m=8, p=P4
        )
        s_all_v2 = s_all[:].rearrange("P (n jl p) -> P n jl p", n=N, p=P4)
        for ib in range(4):
            m2 = pm2.tile([128, N * J * P4], f16, tag="m2")
            m2_v = m2[:].rearrange("P (n j p) -> P n j p", n=N, p=P4)
            cb = (
                c_sb_v[:, ib]
                .rearrange("P n (o p) -> P n o p", o=1)
                .broadcast_to([128, N, J, P4])
            )
            nc.vector.tensor_tensor(m2_v, u2_v[:, ib], cb, op=OP.mult)
            for jq in range(4):
                off = 6 - jq * 2
                nc.tensor.matmul(
                    s_all_v2,
                    ws_t[:, off : off + 128],
                    m2_v[:, :, jq * 4 : (jq + 1) * 4],
                    start=(ib == 0 and jq == 0),
                    stop=(ib == 3 and jq == 3),
                    skip_group_check=True,
                )

        # ---- squash ----
        # ssq = s^2 (bf16 keeps fp32 range; fp16 would flush subnormals)
        ssq = psq.tile([128, 4 * 8 * 4 * P4], bf16, tag="ssq")
        nc.scalar.activation(ssq[:], s_all[:], AF.Square)
        ssq_v = ssq[:].rearrange("P (q m jl p) -> P q m jl p", q=4, m=8, p=P4)
        t1 = psq.tile([128, 4 * 8 * 2 * P4], bf16, tag="t1")
        t1_v = t1[:].rearrange("P (q m jl p) -> P q m jl p", q=4, m=8, p=P4)
        nc.gpsimd.tensor_tensor(
            t1_v, ssq_v[:, :, :, 0:2], ssq_v[:, :, :, 2:4], op=OP.add
        )
        ssq_l = psq.tile([128, 4 * 8 * P4], bf16, tag="ssql")
        nc.vector.tensor_tensor(
            ssq_l[:].rearrange("P (q m p) -> P q m p", q=4, p=P4),
            t1_v[:, :, :, 0],
            t1_v[:, :, :, 1],
            op=OP.add,
        )
        # n2[(g,x8), (nq,m,p4)] = sum_j s^2 via PE partition contraction
        n2 = pnps.tile([128, 4 * 8 * P4], f32, tag="n2")
        nc.tensor.matmul(n2[:], wn_t[:], ssq_l[:], start=True, stop=True)
        # clamp away 0 so ln is finite; out is ~0 there anyway
        n2c = psq.tile([128, 4 * 8 * P4], f32, tag="n2c")
        nc.gpsimd.tensor_scalar(n2c[:], n2[:], 1e-30, None, op0=OP.max)
        lnn = psq.tile([128, 4 * 8 * P4], f32, tag="lnn")
        nc.scalar.activation(lnn[:], n2c[:], AF.Ln)
        # r = exp(.5 ln n2) = |s|; rn = exp(-.5 ln n2) = 1/|s|
        r_t = psq.tile([128, 4 * 8 * P4], f32, tag="r")
        nc.scalar.activation(r_t[:], lnn[:], AF.Exp, scale=0.5)
        rn_t = psq.tile([128, 4 * 8 * P4], f32, tag="rn")
        nc.scalar.activation(rn_t[:], lnn[:], AF.Exp, scale=-0.5)
        en_t = psq.tile([128, 4 * 8 * P4], f32, tag="en")
        nc.scalar.activation(en_t[:], r_t[:], AF.Exp, scale=-1.0)
        g_t = psq.tile([128, 4 * 8 * P4], f32, tag="g")
        nc.vector.scalar_tensor_tensor(
            g_t[:], en_t[:], 1.0, rn_t[:], op0=OP.subtract, op1=OP.mult
        )  # g = (en - 1) / r
        g_b = (
            g_t[:]
            .rearrange("P (q m o p) -> P q m o p", q=4, m=8, o=1)
            .broadcast_to([128, 4, 8, 4, P4])
        )

        outt = pout.tile([128, 4 * 8 * 4 * P4], f16, tag="outt")
        nc.gpsimd.scalar_tensor_tensor(
            outt[:].rearrange("P (q m jl p) -> P q m jl p", q=4, m=8, p=P4),
            s_all_v,
            -1.0,
            g_b,
            op0=OP.mult,
            op1=OP.mult,
        )  # out = (-s) * g = s * (1-en)/r

        # only the r=0 replicas carry data: 64 partitions, stride 2.
        # Issue from the Pool queue: outt is Pool-produced (no queue stall)
        # and SP stays free for the next block's loads.
        nc.gpsimd.dma_start(o_d[blk], outt[::2, :])


def round_f16(x):
    return x.astype(np.float16)


def encode_u1(shard):
    """[I, N, J, pix] -> [blk, (j,il)=128, (ib,n,p64)] fp16."""
    a = shard.reshape(4, 8, N, J, NBLK, BLK)          # ib, il, n, j, blk, p
    # -> blk, j, il, ib, n, p
    return np.ascontiguousarray(
        a.transpose(4, 3, 1, 0, 2, 5)
    ).astype(np.float16)


def encode_u2(shard):
    """[I, N, J, pix] -> [blk, (g,il)=128, (ib,n,j,p4)] fp16."""
    a = shard.reshape(4, 8, N, J, NBLK, NG, P4)       # ib, il, n, j, blk, g, p4
    # -> blk, g, il, ib, n, j, p4
    return np.ascontiguousarray(
        a.transpose(4, 5, 1, 0, 2, 3, 6)
    ).astype(np.float16)


def decode_out(arr):
    """[blk, 64=(g,jq), (nq,m8,jl4,p4)] fp16 -> [N, J, pix] f32.

    n = nq*8+m; j = jq*4+jl; pixel = blk*64 + g*4 + p
    """
    a = arr.astype(np.float32).reshape(NBLK, NG, 4, 4, 8, 4, P4)
    # dims: blk, g, jq, nq, m, jl, p -> (nq,m), (jq,jl), (blk,g,p)
    return np.ascontiguousarray(a.transpose(3, 4, 2, 5, 0, 1, 6)).reshape(
        N, J, PIX
    )


_CACHE = {}


def _patch_act_tables():
    """Keep only natural_log_exp_and_others (Copy/Exp/Ln/Square): every
    function this kernel uses lives in one table, so exactly ONE
    LoadActFuncSet is emitted. Other set entries are kept (emptied) to
    preserve act_func_set_id indices."""
    if getattr(bacc, "_ant_act_tables_patched", False):
        return
    real = bacc.get_activation_tables

    def patched(module_arch):
        tabs = real(module_arch)
        keep = {"natural_log_exp_and_others"}
        return {
            name: (fns if name in keep else set()) for name, fns in tabs.items()
        }

    bacc.get_activation_tables = patched
    bacc._ant_act_tables_patched = True


def _get_program(with_b=False):
    key = with_b
    if key in _CACHE:
        return _CACHE[key]
    _patch_act_tables()
    nc = bacc.Bacc("TRN2", target_bir_lowering=False, debug=False)
    aps = {}
    aps["u1"] = nc.dram_tensor(
        "u1", [NBLK, 128, 4 * N * BLK], f16, kind="ExternalInput"
    ).ap()
    aps["u2"] = nc.dram_tensor(
        "u2", [NBLK, 128, 4 * N * J * P4], f16, kind="ExternalInput"
    ).ap()
    wts = _build_weight_arrays()
    aps["wv"] = nc.dram_tensor("wv", [128, 128], f16, kind="ExternalInput").ap()
    aps["wc_band"] = nc.dram_tensor(
        "wc_band", [128, 248], f16, kind="ExternalInput"
    ).ap()
    aps["ws_band"] = nc.dram_tensor(
        "ws_band", [128, 134], f16, kind="ExternalInput"
    ).ap()
    aps["wn"] = nc.dram_tensor("wn", [128, 128], f16, kind="ExternalInput").ap()
    if with_b:
        aps["bt"] = nc.dram_tensor(
            "bt", [128, 4 * N * P4], f32, kind="ExternalInput"
        ).ap()
    aps["out"] = nc.dram_tensor(
        "out", [NBLK, 64, 4 * 8 * 4 * P4], f16, kind="ExternalOutput"
    ).ap()

    with tile.TileContext(nc) as tc:
        with ExitStack() as ctx:
            _emit(ctx, tc, aps, with_b)
    nc.compile()

    _CACHE[key] = (nc, wts)
    return _CACHE[key]


def kernel(u: np.ndarray, b: np.ndarray) -> np.ndarray:
    u = np.asarray(u, dtype=np.float32)
    b = np.asarray(b, dtype=np.float32)
    with_b = bool(np.any(b))
    nc, wts = _get_program(with_b=with_b)

    base = {
        "wv": wts["wv"].astype(np.float16),
        "wc_band": wts["wc_band"].astype(np.float16),
        "ws_band": wts["ws_band"].astype(np.float16),
        "wn": wts["wn"].astype(np.float16),
    }
    if with_b:
        base["bt"] = _b_tile_array(b)
    in_maps = []
    for c in range(NCORES):
        bb = c // 2
        h0 = 16 * (c % 2)
        shard = u[bb, :, :, :, h0 : h0 + 16, :].reshape(I, N, J, PIX)
        m = dict(base)
        m["u1"] = encode_u1(shard)
        m["u2"] = encode_u2(shard)
        in_maps.append(m)

    res = run_bass_kernel_spmd(nc, in_maps, core_ids=list(range(NCORES)))
    out = np.zeros((B, N, J, H, W), np.float32)
    for c in range(NCORES):
        bb = c // 2
        h0 = 16 * (c % 2)
        out[bb, :, :, h0 : h0 + 16, :] = decode_out(res.results[c]["out"]).reshape(
            N, J, 16, W
        )
    return out


# revision 25
# speedup vs baseline: 1.0229x; 1.0229x over previous
"""Trainium2 Bass kernel for capsule attention-routing.

Reference computation (per pixel; 4096 independent problems of shape
[I=32 in-caps, N=32 out-caps, J=16 caps-dim]):
    v[n,j]   = sum_i u[i,n,j]
    cp[i,n]  = sum_j u[i,n,j] * v[n,j] / 4
    c[i,n]   = softmax_n(cp)[i,n] + b[i,n]
    s[n,j]   = sum_i u[i,n,j] * c[i,n]
    out[n,j] = (1 - exp(-|s|_j)) * s[n,j] / |s|_j

Sharding: data-parallel over (batch, h-half): 8 cores x 512 pixels.

Per-core strategy (dual layout, u streamed twice from HBM as fp16):
  L1 (j-major): partitions (j*8+il), free (ib, n, p64)  [il=i%8, i=ib*8+il]
     - v-pass: PE contracts il (+PSUM accum over ib), broadcast over rows
     - c-mult: DVE 2x-mode fp16 w = u1 * v
     - c-red : PE contracts j via banded 0.25-delta weights ->
               cp[(g*8+il) parts, (ib,n,p4)]  [g = pixel>>2]
  softmax over n on the small cp tile (Act exp f32, Pool z-reduce,
  DVE reciprocal+mult -> c_sb fp16)
  L2 (i-major): partitions (g*8+il), free (ib, n, j, p4)
     - s-mult: DVE 2x-mode m2 = u2 * broadcast_j(c_sb)  (no PE broadcast,
               no PSUM->SBUF copies: c broadcasts via a stride-0 free dim)
     - s-red : PE contracts il within g strips (+accum over ib) ->
               s[(g,x8-replicated) parts, (n8,j,p4)] in 4 nq PSUM banks
  squash: Act square, Pool j-reduce, r = exp(.5*ln(n2)) / rn = exp(-.5*ln n2)
  (single act table: no LoadActFuncSet flips), Pool final multiply.
Softmax runs without max-subtraction (|cp| <~ 45 is safe in fp32 exp).
EPS=1e-20 is negligible: 1-1/(exp(r)+eps) == 1-exp(-r), 1/(r+eps) == 1/r.
"""

import numpy as np
from contextlib import ExitStack

import concourse.bass as bass
import concourse.bacc as bacc
import concourse.tile as tile
import concourse.mybir as mybir
from concourse.bass_utils import run_bass_kernel_spmd

dt = mybir.dt
AF = mybir.ActivationFunctionType
OP = mybir.AluOpType

B, I, N, J, H, W = 4, 32, 32, 16, 32, 32
HW = H * W
NCORES = 8
PIX = B * HW // NCORES      # 512 pixels per core
BLK = 64                    # pixels per block
NBLK = PIX // BLK           # 8
NG = 16                     # pixel groups of 4 per block (g = pixel>>2)
P4 = 4
SCALE = 0.25                # 1/sqrt(16)

f32, bf16, f16 = dt.float32, dt.bfloat16, dt.float16


def _build_weight_arrays():
    il_of = np.arange(128) % 8          # L1 partition -> il is p%8? no: p=(j,il)
    # L1 partitions: p = j*8 + il  -> j = p//8, il = p%8
    j_of = np.arange(128) // 8
    il1 = np.arange(128) % 8

    # v-pass: out[(j2,il2)] = sum_il u[(j,il)] for j==j2 (broadcast over il2)
    wv = np.zeros((128, 128), np.float32)
    for p_in in range(128):
        for p_out in range(128):
            if j_of[p_in] == j_of[p_out]:
                wv[p_in, p_out] = 1.0

    # c-red band: window at offset off(g)=2*(120 - g*8) bytes gives the
    # [128,128] weight mapping (j,il) -> out partition (g*8+il), scaled 0.25.
    # band[(j,il), c] = 0.25 iff c == 120 + il
    wc_band = np.zeros((128, 248), np.float32)
    for p_in in range(128):
        wc_band[p_in, 120 + il1[p_in]] = SCALE

    # s-red band: window at offset off(jq)=2*(6 - jq*2) bytes maps L2
    # partitions (g,il) -> out partition (g*8 + jq*2 + r), r=0,1 replicas.
    # band[(g,il), c] = 1 iff c in (g*8+6, g*8+7)
    ws_band = np.zeros((128, 134), np.float32)
    g_of = np.arange(128) // 8
    for p_in in range(128):
        ws_band[p_in, g_of[p_in] * 8 + 6] = 1.0
        ws_band[p_in, g_of[p_in] * 8 + 7] = 1.0

    # n2: contract the 8 rows of each g strip (each real value appears
    # twice via the r2 replicas -> 0.5)
    wn = np.zeros((128, 128), np.float32)
    for p_in in range(128):
        for p_out in range(128):
            if p_in // 8 == p_out // 8:
                wn[p_in, p_out] = 0.5

    return {"wv": wv, "wc_band": wc_band, "ws_band": ws_band, "wn": wn}


def _b_tile_array(b_np):
    # bt[(g*8+il), (ib, n, p4)] = b[ib*8+il, n]
    bt = np.zeros((128, 4 * N * P4), np.float32)
    bsl = np.asarray(b_np).reshape(I, N)
    for g in range(NG):
        for il in range(8):
            row = g * 8 + il
            for ib in range(4):
                for n in range(N):
                    bt[row, (ib * N + n) * P4 : (ib * N + n + 1) * P4] = bsl[
                        ib * 8 + il, n
                    ]
    return bt


def _emit(ctx: ExitStack, tc: tile.TileContext, aps: dict, with_b: bool):
    nc = tc.nc
    u1_d, u2_d, o_d = aps["u1"], aps["u2"], aps["out"]

    # constants
    pconst = ctx.enter_context(tc.tile_pool(name="const", bufs=1))
    wv_t = pconst.tile([128, 128], f16, tag="wv")
    wcb_t = pconst.tile([128, 248], f16, tag="wcb")
    ws_t = pconst.tile([128, 134], f16, tag="ws")
    wn_t = pconst.tile([128, 128], f16, tag="wn")
    nc.sync.dma_start(wv_t[:], aps["wv"])
    nc.sync.dma_start(wcb_t[:], aps["wc_band"])
    nc.sync.dma_start(ws_t[:], aps["ws_band"])
    nc.sync.dma_start(wn_t[:], aps["wn"])
    bt_t = None
    if with_b:
        bt_t = pconst.tile([128, 4 * N * P4], f32, tag="bt")
        nc.sync.dma_start(bt_t[:], aps["bt"])

    # pools
    pu1 = ctx.enter_context(tc.tile_pool(name="u1", bufs=2))
    pu2 = ctx.enter_context(tc.tile_pool(name="u2", bufs=3))
    pw1 = ctx.enter_context(tc.tile_pool(name="w1", bufs=2))
    pm2 = ctx.enter_context(tc.tile_pool(name="m2", bufs=3))
    pvsb = ctx.enter_context(tc.tile_pool(name="vsb", bufs=2))
    pce = ctx.enter_context(tc.tile_pool(name="ce", bufs=2))
    pcsb = ctx.enter_context(tc.tile_pool(name="csb", bufs=2))
    psq = ctx.enter_context(tc.tile_pool(name="sq", bufs=2))
    pout = ctx.enter_context(tc.tile_pool(name="out", bufs=2))

    pvps = ctx.enter_context(tc.tile_pool(name="vps", bufs=2, space="PSUM"))
    pcps = ctx.enter_context(tc.tile_pool(name="cps", bufs=2, space="PSUM"))
    psps = ctx.enter_context(tc.tile_pool(name="sps", bufs=3, space="PSUM"))
    pnps = ctx.enter_context(tc.tile_pool(name="nps", bufs=1, space="PSUM"))

    def load(blk):
        u1 = pu1.tile([128, 4 * N * BLK], f16, tag="u1")   # [(j,il),(ib,n,p64)]
        nc.sync.dma_start(u1[:], u1_d[blk])
        u2 = pu2.tile([128, 4 * N * J * P4], f16, tag="u2")  # [(g,il),(ib,n,j,p4)]
        nc.sync.dma_start(u2[:], u2_d[blk])
        return u1, u2

    loads = {0: load(0)}

    def front(blk):
        """loads -> v-pass -> c-mult -> c-red -> softmax -> c_sb."""
        if blk + 1 < NBLK:
            loads[blk + 1] = load(blk + 1)
        u1, u2 = loads.pop(blk)

        u1_v = u1[:].rearrange("P (ib n p) -> P ib n p", ib=4, p=BLK)

        # ---- v-pass (PE): v[(j,il-bcast),(n,p64)] = sum_i u1 ----
        v_sb = pvsb.tile([128, N * BLK], f16, tag="vsb")
        v_sb_v = v_sb[:].rearrange("P (n p) -> P n p", p=BLK)
        for st in range(4):
            v_ps = pvps.tile([128, 512], f32, tag="vps")
            v_ps_v = v_ps[:].rearrange("P (n p) -> P n p", p=16)
            for ib in range(4):
                nc.tensor.matmul(
                    v_ps_v,
                    wv_t[:],
                    u1_v[:, ib, :, st * 16 : (st + 1) * 16],
                    start=(ib == 0),
                    stop=(ib == 3),
                )
            nc.gpsimd.tensor_scalar(
                v_sb_v[:, :, st * 16 : (st + 1) * 16], v_ps_v, 0.0, None,
                op0=OP.add,
            )

        # ---- c-mult (DVE 2x): w1 = u1 * v ----
        w1 = pw1.tile([128, 4 * N * BLK], f16, tag="w1")
        w1_v = w1[:].rearrange("P (ib n p) -> P ib n p", ib=4, p=BLK)
        for ib in range(4):
            nc.vector.tensor_tensor(
                w1_v[:, ib], u1_v[:, ib], v_sb_v, op=OP.mult
            )

        # ---- c-red (PE): cp[(g,il), (ib,n,p4)] = 0.25*sum_j w1 ----
        cp = pcps.tile([128, 4 * N * P4], f32, tag="cp")
        cp_v = cp[:].rearrange("P (ib n p) -> P ib n p", ib=4, p=P4)
        for g in range(NG):
            off = 120 - g * 8
            nc.tensor.matmul(
                cp_v,
                wcb_t[:, off : off + 128],
                w1_v[:, :, :, g * P4 : (g + 1) * P4],
                start=(g == 0),
                stop=(g == NG - 1),
                skip_group_check=True,
            )

        # ---- softmax over n (no max-subtraction) ----
        c_e = pce.tile([128, 4 * N * P4], f32, tag="ce")
        nc.scalar.activation(c_e[:], cp[:], AF.Exp)
        c_e_v = c_e[:].rearrange("P (ib n p) -> P ib n p", ib=4, p=P4)
        z = pcsb.tile([128, 4 * P4], f32, tag="z")
        nc.vector.tensor_reduce(
            z[:].rearrange("P (ib p) -> P ib p", ib=4),
            c_e[:].rearrange("P (ib n p) -> P ib p n", ib=4, p=P4),
            axis=mybir.AxisListType.X,
            op=OP.add,
        )
        rz = pcsb.tile([128, 4 * P4], f32, tag="rz")
        nc.vector.reciprocal(rz[:], z[:])
        rz_b = (
            rz[:]
            .rearrange("P (ib o p) -> P ib o p", ib=4, o=1)
            .broadcast_to([128, 4, N, P4])
        )
        c_sb = pcsb.tile([128, 4 * N * P4], f16, tag="csb")
        c_sb_v = c_sb[:].rearrange("P (ib n p) -> P ib n p", ib=4, p=P4)
        if with_b:
            c_f = pcsb.tile([128, 4 * N * P4], f32, tag="cf")
            nc.gpsimd.tensor_tensor(
                c_f[:].rearrange("P (ib n p) -> P ib n p", ib=4, p=P4),
                c_e_v,
                rz_b,
                op=OP.mult,
            )
            nc.gpsimd.tensor_tensor(c_sb[:], c_f[:], bt_t[:], op=OP.add)
        else:
            nc.gpsimd.tensor_tensor(c_sb_v, c_e_v, rz_b, op=OP.mult)
        return u2, c_sb

    def back(blk, u2, c_sb):
        """m2 -> s-red -> squash -> store."""
        c_sb_v = c_sb[:].rearrange("P (ib n p) -> P ib n p", ib=4, p=P4)
        # s_all[(g, jq, r2) parts, (nq, m8, jl4, p4)]; j = jq*4 + jl
        u2_v = u2[:].rearrange("P (ib n j p) -> P ib n j p", ib=4, n=N, p=P4)
        s_all = psps.tile([128, 4 * 8 * 4 * P4], f32, tag="sall")
        s_all_v = s_all[:].rearrange(
            "P (q m jl p) -> P q m jl p", q=4, m=8, p=P4
        )
        s_all_v2 = s_all[:].rearrange("P (n jl p) -> P n jl p", n=N, p=P4)
        for ib in range(4):
            m2 = pm2.tile([128, N * J * P4], f16, tag="m2")
            m2_v = m2[:].rearrange("P (n j p) -> P n j p", n=N, p=P4)
            cb = (
                c_sb_v[:, ib]
                .rearrange("P n (o p) -> P n o p", o=1)
                .broadcast_to([128, N, J, P4])
            )
            nc.vector.tensor_tensor(m2_v, u2_v[:, ib], cb, op=OP.mult)
            for jq in range(4):
                off = 6 - jq * 2
                nc.tensor.matmul(
                    s_all_v2,
                    ws_t[:, off : off + 128],
                    m2_v[:, :, jq * 4 : (jq + 1) * 4],
                    start=(ib == 0 and jq == 0),
                    stop=(ib == 3 and jq == 3),
                    skip_group_check=True,
                )

        # ---- squash ----
        # ssq = s^2 (bf16 keeps fp32 range; fp16 would flush subnormals)
        ssq = psq.tile([128, 4 * 8 * 4 * P4], bf16, tag="ssq")
        nc.scalar.activation(ssq[:], s_all[:], AF.Square)
        ssq_v = ssq[:].rearrange("P (q m jl p) -> P q m jl p", q=4, m=8, p=P4)
        t1 = psq.tile([128, 4 * 8 * 2 * P4], bf16, tag="t1")
        t1_v = t1[:].rearrange("P (q m jl p) -> P q m jl p", q=4, m=8, p=P4)
        nc.gpsimd.tensor_tensor(
            t1_v, ssq_v[:, :, :, 0:2], ssq_v[:, :, :, 2:4], op=OP.add
        )
        ssq_l = psq.tile([128, 4 * 8 * P4], bf16, tag="ssql")
        nc.vector.tensor_tensor(
            ssq_l[:].rearrange("P (q m p) -> P q m p", q=4, p=P4),
            t1_v[:, :, :, 0],
            t1_v[:, :, :, 1],
            op=OP.add,
        )
        # n2[(g,x8), (nq,m,p4)] = sum_j s^2 via PE partition contraction
        n2 = pnps.tile([128, 4 * 8 * P4], f32, tag="n2")
        nc.tensor.matmul(n2[:], wn_t[:], ssq_l[:], start=True, stop=True)
        # clamp away 0 so ln is finite; out is ~0 there anyway
        n2c = psq.tile([128, 4 * 8 * P4], f32, tag="n2c")
        nc.gpsimd.tensor_scalar(n2c[:], n2[:], 1e-30, None, op0=OP.max)
        lnn = psq.tile([128, 4 * 8 * P4], f32, tag="lnn")
        nc.scalar.activation(lnn[:], n2c[:], AF.Ln)
        # r = exp(.5 ln n2) = |s|; rn = exp(-.5 ln n2) = 1/|s|
        r_t = psq.tile([128, 4 * 8 * P4], f32, tag="r")
        nc.scalar.activation(r_t[:], lnn[:], AF.Exp, scale=0.5)
        rn_t = psq.tile([128, 4 * 8 * P4], f32, tag="rn")
        nc.scalar.activation(rn_t[:], lnn[:], AF.Exp, scale=-0.5)
        en_t = psq.tile([128, 4 * 8 * P4], f32, tag="en")
        nc.scalar.activation(en_t[:], r_t[:], AF.Exp, scale=-1.0)
        g_t = psq.tile([128, 4 * 8 * P4], f32, tag="g")
        nc.vector.scalar_tensor_tensor(
            g_t[:], en_t[:], 1.0, rn_t[:], op0=OP.subtract, op1=OP.mult
        )  # g = (en - 1) / r
        g_b = (
            g_t[:]
            .rearrange("P (q m o p) -> P q m o p", q=4, m=8, o=1)
            .broadcast_to([128, 4, 8, 4, P4])
        )

        outt = pout.tile([128, 4 * 8 * 4 * P4], f16, tag="outt")
        nc.gpsimd.scalar_tensor_tensor(
            outt[:].rearrange("P (q m jl p) -> P q m jl p", q=4, m=8, p=P4),
            s_all_v,
            -1.0,
            g_b,
            op0=OP.mult,
            op1=OP.mult,
        )  # out = (-s) * g = s * (1-en)/r

        # only the r=0 replicas carry data: 64 partitions, stride 2
        nc.sync.dma_start(o_d[blk], outt[::2, :])

    # 2-stage software pipeline: emit front(k+1) before back(k) so each
    # in-order engine queue sees work in expected-ready order (otherwise
    # back(k)'s tail ops head-of-line block front(k+1), idling the PE).
    pend = {}
    for blk in range(NBLK + 1):
        if blk < NBLK:
            pend[blk] = front(blk)
        if blk >= 1:
            back(blk - 1, *pend.pop(blk - 1))


def round_f16(x):
    return x.astype(np.float16)


def encode_u1(shard):
    """[I, N, J, pix] -> [blk, (j,il)=128, (ib,n,p64)] fp16."""
    a = shard.reshape(4, 8, N, J, NBLK, BLK)          # ib, il, n, j, blk, p
    # -> blk, j, il, ib, n, p
    return np.ascontiguousarray(
        a.transpose(4, 3, 1, 0, 2, 5)
    ).astype(np.float16)


def encode_u2(shard):
    """[I, N, J, pix] -> [blk, (g,il)=128, (ib,n,j,p4)] fp16."""
    a = shard.reshape(4, 8, N, J, NBLK, NG, P4)       # ib, il, n, j, blk, g, p4
    # -> blk, g, il, ib, n, j, p4
    return np.ascontiguousarray(
        a.transpose(4, 5, 1, 0, 2, 3, 6)
    ).astype(np.float16)


def decode_out(arr):
    """[blk, 64=(g,jq), (nq,m8,jl4,p4)] fp16 -> [N, J, pix] f32.

    n = nq*8+m; j = jq*4+jl; pixel = blk*64 + g*4 + p
    """
    a = arr.astype(np.float32).reshape(NBLK, NG, 4, 4, 8, 4, P4)
    # dims: blk, g, jq, nq, m, jl, p -> (nq,m), (jq,jl), (blk,g,p)
    return np.ascontiguousarray(a.transpose(3, 4, 2, 5, 0, 1, 6)).reshape(
        N, J, PIX
    )


_CACHE = {}


def _patch_act_tables():
    """Keep only natural_log_exp_and_others (Copy/Exp/Ln/Square): every
    function this kernel uses lives in one table, so exactly ONE
    LoadActFuncSet is emitted. Other set entries are kept (emptied) to
    preserve act_func_set_id indices."""
    if getattr(bacc, "_ant_act_tables_patched", False):
        return
    real = bacc.get_activation_tables

    def patched(module_arch):
        tabs = real(module_arch)
        keep = {"natural_log_exp_and_others"}
        return {
            name: (fns if name in keep else set()) for name, fns in tabs.items()
        }

    bacc.get_activation_tables = patched
    bacc._ant_act_tables_patched = True


def _get_program(with_b=False):
    key = with_b
    if key in _CACHE:
        return _CACHE[key]
    _patch_act_tables()
    nc = bacc.Bacc("TRN2", target_bir_lowering=False, debug=False)
    aps = {}
    aps["u1"] = nc.dram_tensor(
        "u1", [NBLK, 128, 4 * N * BLK], f16, kind="ExternalInput"
    ).ap()
    aps["u2"] = nc.dram_tensor(
        "u2", [NBLK, 128, 4 * N * J * P4], f16, kind="ExternalInput"
    ).ap()
    wts = _build_weight_arrays()
    aps["wv"] = nc.dram_tensor("wv", [128, 128], f16, kind="ExternalInput").ap()
    aps["wc_band"] = nc.dram_tensor(
        "wc_band", [128, 248], f16, kind="ExternalInput"
    ).ap()
    aps["ws_band"] = nc.dram_tensor(
        "ws_band", [128, 134], f16, kind="ExternalInput"
    ).ap()
    aps["wn"] = nc.dram_tensor("wn", [128, 128], f16, kind="ExternalInput").ap()
    if with_b:
        aps["bt"] = nc.dram_tensor(
            "bt", [128, 4 * N * P4], f32, kind="ExternalInput"
        ).ap()
    aps["out"] = nc.dram_tensor(
        "out", [NBLK, 64, 4 * 8 * 4 * P4], f16, kind="ExternalOutput"
    ).ap()

    with tile.TileContext(nc) as tc:
        with ExitStack() as ctx:
            _emit(ctx, tc, aps, with_b)
    nc.compile()

    _CACHE[key] = (nc, wts)
    return _CACHE[key]


def kernel(u: np.ndarray, b: np.ndarray) -> np.ndarray:
    u = np.asarray(u, dtype=np.float32)
    b = np.asarray(b, dtype=np.float32)
    with_b = bool(np.any(b))
    nc, wts = _get_program(with_b=with_b)

    base = {
        "wv": wts["wv"].astype(np.float16),
        "wc_band": wts["wc_band"].astype(np.float16),
        "ws_band": wts["ws_band"].astype(np.float16),
        "wn": wts["wn"].astype(np.float16),
    }
    if with_b:
        base["bt"] = _b_tile_array(b)
    in_maps = []
    for c in range(NCORES):
        bb = c // 2
        h0 = 16 * (c % 2)
        shard = u[bb, :, :, :, h0 : h0 + 16, :].reshape(I, N, J, PIX)
        m = dict(base)
        m["u1"] = encode_u1(shard)
        m["u2"] = encode_u2(shard)
        in_maps.append(m)

    res = run_bass_kernel_spmd(nc, in_maps, core_ids=list(range(NCORES)))
    out = np.zeros((B, N, J, H, W), np.float32)
    for c in range(NCORES):
        bb = c // 2
        h0 = 16 * (c % 2)
        out[bb, :, :, h0 : h0 + 16, :] = decode_out(res.results[c]["out"]).reshape(
            N, J, 16, W
        )
    return out


# revision 32
# speedup vs baseline: 1.2835x; 1.2548x over previous
"""Trainium2 Bass kernel for capsule attention-routing.

Reference computation (per pixel; 4096 independent problems of shape
[I=32 in-caps, N=32 out-caps, J=16 caps-dim]):
    v[n,j]   = sum_i u[i,n,j]
    cp[i,n]  = sum_j u[i,n,j] * v[n,j] / 4
    c[i,n]   = softmax_n(cp)[i,n] + b[i,n]
    s[n,j]   = sum_i u[i,n,j] * c[i,n]
    out[n,j] = (1 - exp(-|s|_j)) * s[n,j] / |s|_j

Sharding: data-parallel over (batch, h-half): 8 cores x 512 pixels.

Per-core strategy (dual layout, u streamed twice from HBM as fp16):
  L1 (j-major): partitions (j*8+il), free (ib, n, p64)  [il=i%8, i=ib*8+il]
     - v-pass: PE contracts il (+PSUM accum over ib), broadcast over rows
     - c-mult: DVE 2x-mode fp16 w = u1 * v
     - c-red : PE contracts j via banded 0.25-delta weights ->
               cp[(g*8+il) parts, (ib,n,p4)]  [g = pixel>>2]
  softmax over n on the small cp tile (Act exp f32, Pool z-reduce,
  DVE reciprocal+mult -> c_sb fp16)
  L2 (i-major): partitions (g*8+il), free (ib, n, j, p4)
     - s-mult: DVE 2x-mode m2 = u2 * broadcast_j(c_sb)  (no PE broadcast,
               no PSUM->SBUF copies: c broadcasts via a stride-0 free dim)
     - s-red : PE contracts il within g strips (+accum over ib) ->
               s[(g,x8-replicated) parts, (n8,j,p4)] in 4 nq PSUM banks
  squash: Act square, Pool j-reduce, r = exp(.5*ln(n2)) / rn = exp(-.5*ln n2)
  (single act table: no LoadActFuncSet flips), Pool final multiply.
Softmax runs without max-subtraction (|cp| <~ 45 is safe in fp32 exp).
EPS=1e-20 is negligible: 1-1/(exp(r)+eps) == 1-exp(-r), 1/(r+eps) == 1/r.
"""

import numpy as np
from contextlib import ExitStack

import concourse.bass as bass
import concourse.bacc as bacc
import concourse.tile as tile
import concourse.mybir as mybir
from concourse.bass_utils import run_bass_kernel_spmd

dt = mybir.dt
AF = mybir.ActivationFunctionType
OP = mybir.AluOpType

B, I, N, J, H, W = 4, 32, 32, 16, 32, 32
HW = H * W
NCORES = 8
PIX = B * HW // NCORES      # 512 pixels per core
BLK = 64                    # pixels per block
NBLK = PIX // BLK           # 8
NG = 16                     # pixel groups of 4 per block (g = pixel>>2)
P4 = 4
SCALE = 0.25                # 1/sqrt(16)

f32, bf16, f16 = dt.float32, dt.bfloat16, dt.float16


def _build_weight_arrays():
    il_of = np.arange(128) % 8          # L1 partition -> il is p%8? no: p=(j,il)
    # L1 partitions: p = j*8 + il  -> j = p//8, il = p%8
    j_of = np.arange(128) // 8
    il1 = np.arange(128) % 8

    # v-pass: out[(j2,il2)] = sum_il u[(j,il)] for j==j2 (broadcast over il2)
    wv = np.zeros((128, 128), np.float32)
    for p_in in range(128):
        for p_out in range(128):
            if j_of[p_in] == j_of[p_out]:
                wv[p_in, p_out] = 1.0

    # c-red band: window at offset off(g)=2*(120 - g*8) bytes gives the
    # [128,128] weight mapping (j,il) -> out partition (g*8+il), scaled 0.25.
    # band[(j,il), c] = 0.25 iff c == 120 + il
    wc_band = np.zeros((128, 248), np.float32)
    for p_in in range(128):
        wc_band[p_in, 120 + il1[p_in]] = SCALE

    # s-red band: window at offset off(jq)=2*(6 - jq*2) bytes maps L2
    # partitions (g,il) -> out partition (g*8 + jq*2 + r), r=0,1 replicas.
    # band[(g,il), c] = 1 iff c in (g*8+6, g*8+7)
    ws_band = np.zeros((128, 134), np.float32)
    g_of = np.arange(128) // 8
    for p_in in range(128):
        ws_band[p_in, g_of[p_in] * 8 + 6] = 1.0
        ws_band[p_in, g_of[p_in] * 8 + 7] = 1.0

    # n2: contract the 8 rows of each g strip (each real value appears
    # twice via the r2 replicas -> 0.5)
    wn = np.zeros((128, 128), np.float32)
    for p_in in range(128):
        for p_out in range(128):
            if p_in // 8 == p_out // 8:
                wn[p_in, p_out] = 0.5

    return {"wv": wv, "wc_band": wc_band, "ws_band": ws_band, "wn": wn}


def _b_tile_array(b_np):
    # bt[(g*8+il), (ib, n, p4)] = b[ib*8+il, n]
    bt = np.zeros((128, 4 * N * P4), np.float32)
    bsl = np.asarray(b_np).reshape(I, N)
    for g in range(NG):
        for il in range(8):
            row = g * 8 + il
            for ib in range(4):
                for n in range(N):
                    bt[row, (ib * N + n) * P4 : (ib * N + n + 1) * P4] = bsl[
                        ib * 8 + il, n
                    ]
    return bt


def _emit(ctx: ExitStack, tc: tile.TileContext, aps: dict, with_b: bool):
    nc = tc.nc
    u1_d, u2_d, o_d = aps["u1"], aps["u2"], aps["out"]

    # constants
    pconst = ctx.enter_context(tc.tile_pool(name="const", bufs=1))
    wv_t = pconst.tile([128, 128], f16, tag="wv")
    wcb_t = pconst.tile([128, 248], f16, tag="wcb")
    ws_t = pconst.tile([128, 134], f16, tag="ws")
    wn_t = pconst.tile([128, 128], f16, tag="wn")
    nc.sync.dma_start(wv_t[:], aps["wv"])
    nc.sync.dma_start(wcb_t[:], aps["wc_band"])
    nc.sync.dma_start(ws_t[:], aps["ws_band"])
    nc.sync.dma_start(wn_t[:], aps["wn"])
    bt_t = None
    if with_b:
        bt_t = pconst.tile([128, 4 * N * P4], f32, tag="bt")
        nc.sync.dma_start(bt_t[:], aps["bt"])

    # pools
    pu1 = ctx.enter_context(tc.tile_pool(name="u1", bufs=3))
    pu2 = ctx.enter_context(tc.tile_pool(name="u2", bufs=2))
    pw1 = ctx.enter_context(tc.tile_pool(name="w1", bufs=2))
    pm2 = ctx.enter_context(tc.tile_pool(name="m2", bufs=3))
    pvsb = ctx.enter_context(tc.tile_pool(name="vsb", bufs=3))
    pce = ctx.enter_context(tc.tile_pool(name="ce", bufs=2))
    pcsb = ctx.enter_context(tc.tile_pool(name="csb", bufs=2))
    psq = ctx.enter_context(tc.tile_pool(name="sq", bufs=2))
    pout = ctx.enter_context(tc.tile_pool(name="out", bufs=2))

    pcsb2 = ctx.enter_context(tc.tile_pool(name="csb2", bufs=3))

    pvps = ctx.enter_context(tc.tile_pool(name="vps", bufs=2, space="PSUM"))
    pcps = ctx.enter_context(tc.tile_pool(name="cps", bufs=2, space="PSUM"))
    psps = ctx.enter_context(tc.tile_pool(name="sps", bufs=3, space="PSUM"))
    pnps = ctx.enter_context(tc.tile_pool(name="nps", bufs=1, space="PSUM"))

    def load(blk):
        u1 = pu1.tile([128, 4 * N * BLK], f16, tag="u1")   # [(j,il),(ib,n,p64)]
        nc.sync.dma_start(u1[:], u1_d[blk])
        u2 = pu2.tile([128, 4 * N * J * P4], f16, tag="u2")  # [(g,il),(ib,n,j,p4)]
        nc.sync.dma_start(u2[:], u2_d[blk])
        return u1, u2

    loads1 = {}
    loads2 = {}

    def load1(blk):
        u1 = pu1.tile([128, 4 * N * BLK], f16, tag="u1")   # [(j,il),(ib,n,p64)]
        nc.sync.dma_start(u1[:], u1_d[blk])
        loads1[blk] = u1

    def load2(blk):
        u2 = pu2.tile([128, 4 * N * J * P4], f16, tag="u2")  # [(g,il),(ib,n,j,p4)]
        nc.sync.dma_start(u2[:], u2_d[blk])
        loads2[blk] = u2

    def s1(blk):
        """v-pass (PE) -> v-copies (Pool); prefetch next u1."""
        if blk + 1 < NBLK:
            load1(blk + 1)
        u1 = loads1.pop(blk)

        u1_v = u1[:].rearrange("P (ib n p) -> P ib n p", ib=4, p=BLK)

        # ---- v-pass (PE): v[(j,il-bcast),(n,p64)] = sum_i u1 ----
        v_sb = pvsb.tile([128, N * BLK], f16, tag="vsb")
        v_sb_v = v_sb[:].rearrange("P (n p) -> P n p", p=BLK)
        for st in range(4):
            v_ps = pvps.tile([128, 512], f32, tag="vps")
            v_ps_v = v_ps[:].rearrange("P (n p) -> P n p", p=16)
            for ib in range(4):
                nc.tensor.matmul(
                    v_ps_v,
                    wv_t[:],
                    u1_v[:, ib, :, st * 16 : (st + 1) * 16],
                    start=(ib == 0),
                    stop=(ib == 3),
                )
            nc.gpsimd.tensor_scalar(
                v_sb_v[:, :, st * 16 : (st + 1) * 16], v_ps_v, 0.0, None,
                op0=OP.add,
            )
        return u1, v_sb

    def s2(blk, u1, v_sb):
        """c-mult (DVE) -> c-red (PE) -> softmax; prefetch u2."""
        load2(blk)
        u1_v = u1[:].rearrange("P (ib n p) -> P ib n p", ib=4, p=BLK)
        v_sb_v = v_sb[:].rearrange("P (n p) -> P n p", p=BLK)

        # ---- c-mult (DVE 2x): w1 = u1 * v ----
        w1 = pw1.tile([128, 4 * N * BLK], f16, tag="w1")
        w1_v = w1[:].rearrange("P (ib n p) -> P ib n p", ib=4, p=BLK)
        for ib in range(4):
            nc.vector.tensor_tensor(
                w1_v[:, ib], u1_v[:, ib], v_sb_v, op=OP.mult
            )

        # ---- c-red (PE): cp[(g,il), (ib,n,p4)] = 0.25*sum_j w1 ----
        cp = pcps.tile([128, 4 * N * P4], f32, tag="cp")
        cp_v = cp[:].rearrange("P (ib n p) -> P ib n p", ib=4, p=P4)
        for g in range(NG):
            off = 120 - g * 8
            nc.tensor.matmul(
                cp_v,
                wcb_t[:, off : off + 128],
                w1_v[:, :, :, g * P4 : (g + 1) * P4],
                start=(g == 0),
                stop=(g == NG - 1),
                skip_group_check=True,
            )

        # ---- softmax over n (no max-subtraction) ----
        c_e = pce.tile([128, 4 * N * P4], f32, tag="ce")
        nc.scalar.activation(c_e[:], cp[:], AF.Exp)
        c_e_v = c_e[:].rearrange("P (ib n p) -> P ib n p", ib=4, p=P4)
        z = pcsb.tile([128, 4 * P4], f32, tag="z")
        nc.vector.tensor_reduce(
            z[:].rearrange("P (ib p) -> P ib p", ib=4),
            c_e[:].rearrange("P (ib n p) -> P ib p n", ib=4, p=P4),
            axis=mybir.AxisListType.X,
            op=OP.add,
        )
        rz = pcsb.tile([128, 4 * P4], f32, tag="rz")
        nc.vector.reciprocal(rz[:], z[:])
        rz_b = (
            rz[:]
            .rearrange("P (ib o p) -> P ib o p", ib=4, o=1)
            .broadcast_to([128, 4, N, P4])
        )
        c_sb = pcsb2.tile([128, 4 * N * P4], f16, tag="csb")
        c_sb_v = c_sb[:].rearrange("P (ib n p) -> P ib n p", ib=4, p=P4)
        if with_b:
            c_f = pcsb.tile([128, 4 * N * P4], f32, tag="cf")
            nc.gpsimd.tensor_tensor(
                c_f[:].rearrange("P (ib n p) -> P ib n p", ib=4, p=P4),
                c_e_v,
                rz_b,
                op=OP.mult,
            )
            nc.gpsimd.tensor_tensor(c_sb[:], c_f[:], bt_t[:], op=OP.add)
        else:
            nc.gpsimd.tensor_tensor(c_sb_v, c_e_v, rz_b, op=OP.mult)
        return c_sb

    def s3(blk, c_sb):
        """m2 -> s-red -> squash -> store."""
        u2 = loads2.pop(blk)
        c_sb_v = c_sb[:].rearrange("P (ib n p) -> P ib n p", ib=4, p=P4)
        # s_all[(g, jq, r2) parts, (nq, m8, jl4, p4)]; j = jq*4 + jl
        u2_v = u2[:].rearrange("P (ib n j p) -> P ib n j p", ib=4, n=N, p=P4)
        s_all = psps.tile([128, 4 * 8 * 4 * P4], f32, tag="sall")
        s_all_v = s_all[:].rearrange(
            "P (q m jl p) -> P q m jl p", q=4, m=8, p=P4
        )
        s_all_v2 = s_all[:].rearrange("P (n jl p) -> P n jl p", n=N, p=P4)
        for ib in range(4):
            m2 = pm2.tile([128, N * J * P4], f16, tag="m2")
            m2_v = m2[:].rearrange("P (n j p) -> P n j p", n=N, p=P4)
            cb = (
                c_sb_v[:, ib]
                .rearrange("P n (o p) -> P n o p", o=1)
                .broadcast_to([128, N, J, P4])
            )
            nc.vector.tensor_tensor(m2_v, u2_v[:, ib], cb, op=OP.mult)
            for jq in range(4):
                off = 6 - jq * 2
                nc.tensor.matmul(
                    s_all_v2,
                    ws_t[:, off : off + 128],
                    m2_v[:, :, jq * 4 : (jq + 1) * 4],
                    start=(ib == 0 and jq == 0),
                    stop=(ib == 3 and jq == 3),
                    skip_group_check=True,
                )

        # ---- squash ----
        # ssq = s^2 (bf16 keeps fp32 range; fp16 would flush subnormals)
        ssq = psq.tile([128, 4 * 8 * 4 * P4], bf16, tag="ssq")
        nc.scalar.activation(ssq[:], s_all[:], AF.Square)
        ssq_v = ssq[:].rearrange("P (q m jl p) -> P q m jl p", q=4, m=8, p=P4)
        t1 = psq.tile([128, 4 * 8 * 2 * P4], bf16, tag="t1")
        t1_v = t1[:].rearrange("P (q m jl p) -> P q m jl p", q=4, m=8, p=P4)
        nc.gpsimd.tensor_tensor(
            t1_v, ssq_v[:, :, :, 0:2], ssq_v[:, :, :, 2:4], op=OP.add
        )
        ssq_l = psq.tile([128, 4 * 8 * P4], bf16, tag="ssql")
        nc.vector.tensor_tensor(
            ssq_l[:].rearrange("P (q m p) -> P q m p", q=4, p=P4),
            t1_v[:, :, :, 0],
            t1_v[:, :, :, 1],
            op=OP.add,
        )
        # n2[(g,x8), (nq,m,p4)] = sum_j s^2 via PE partition contraction
        n2 = pnps.tile([128, 4 * 8 * P4], f32, tag="n2")
        nc.tensor.matmul(n2[:], wn_t[:], ssq_l[:], start=True, stop=True)
        # clamp away 0 so ln is finite; out is ~0 there anyway
        n2c = psq.tile([128, 4 * 8 * P4], f32, tag="n2c")
        nc.gpsimd.tensor_scalar(n2c[:], n2[:], 1e-30, None, op0=OP.max)
        lnn = psq.tile([128, 4 * 8 * P4], f32, tag="lnn")
        nc.scalar.activation(lnn[:], n2c[:], AF.Ln)
        # r = exp(.5 ln n2) = |s|; rn = exp(-.5 ln n2) = 1/|s|
        r_t = psq.tile([128, 4 * 8 * P4], f32, tag="r")
        nc.scalar.activation(r_t[:], lnn[:], AF.Exp, scale=0.5)
        rn_t = psq.tile([128, 4 * 8 * P4], f32, tag="rn")
        nc.scalar.activation(rn_t[:], lnn[:], AF.Exp, scale=-0.5)
        en_t = psq.tile([128, 4 * 8 * P4], f32, tag="en")
        nc.scalar.activation(en_t[:], r_t[:], AF.Exp, scale=-1.0)
        g_t = psq.tile([128, 4 * 8 * P4], f32, tag="g")
        nc.vector.scalar_tensor_tensor(
            g_t[:], en_t[:], 1.0, rn_t[:], op0=OP.subtract, op1=OP.mult
        )  # g = (en - 1) / r
        g_b = (
            g_t[:]
            .rearrange("P (q m o p) -> P q m o p", q=4, m=8, o=1)
            .broadcast_to([128, 4, 8, 4, P4])
        )

        outt = pout.tile([128, 4 * 8 * 4 * P4], f16, tag="outt")
        nc.gpsimd.scalar_tensor_tensor(
            outt[:].rearrange("P (q m jl p) -> P q m jl p", q=4, m=8, p=P4),
            s_all_v,
            -1.0,
            g_b,
            op0=OP.mult,
            op1=OP.mult,
        )  # out = (-s) * g = s * (1-en)/r

        # only the r=0 replicas carry data: 64 partitions, stride 2
        nc.sync.dma_start(o_d[blk], outt[::2, :])

    # 3-stage software pipeline: emit s1(k), s2(k-1), s3(k-2) per iteration
    # so each in-order engine queue sees work in expected-ready order (the
    # PE queue in particular becomes v(k), c-red(k-1), s-red(k-2), each of
    # whose inputs is already in flight — PE stays continuously busy and at
    # full p-state).
    p1, p2 = {}, {}
    load1(0)
    for it in range(NBLK + 2):
        if it < NBLK:
            p1[it] = s1(it)
        if 1 <= it <= NBLK:
            p2[it - 1] = s2(it - 1, *p1.pop(it - 1))
        if it >= 2:
            s3(it - 2, p2.pop(it - 2))


def round_f16(x):
    return x.astype(np.float16)


def encode_u1(shard):
    """[I, N, J, pix] -> [blk, (j,il)=128, (ib,n,p64)] fp16."""
    a = shard.reshape(4, 8, N, J, NBLK, BLK)          # ib, il, n, j, blk, p
    # -> blk, j, il, ib, n, p
    return np.ascontiguousarray(
        a.transpose(4, 3, 1, 0, 2, 5)
    ).astype(np.float16)


def encode_u2(shard):
    """[I, N, J, pix] -> [blk, (g,il)=128, (ib,n,j,p4)] fp16."""
    a = shard.reshape(4, 8, N, J, NBLK, NG, P4)       # ib, il, n, j, blk, g, p4
    # -> blk, g, il, ib, n, j, p4
    return np.ascontiguousarray(
        a.transpose(4, 5, 1, 0, 2, 3, 6)
    ).astype(np.float16)


def decode_out(arr):
    """[blk, 64=(g,jq), (nq,m8,jl4,p4)] fp16 -> [N, J, pix] f32.

    n = nq*8+m; j = jq*4+jl; pixel = blk*64 + g*4 + p
    """
    a = arr.astype(np.float32).reshape(NBLK, NG, 4, 4, 8, 4, P4)
    # dims: blk, g, jq, nq, m, jl, p -> (nq,m), (jq,jl), (blk,g,p)
    return np.ascontiguousarray(a.transpose(3, 4, 2, 5, 0, 1, 6)).reshape(
        N, J, PIX
    )


_CACHE = {}


def _patch_act_tables():
    """Keep only natural_log_exp_and_others (Copy/Exp/Ln/Square): every
    function this kernel uses lives in one table, so exactly ONE
    LoadActFuncSet is emitted. Other set entries are kept (emptied) to
    preserve act_func_set_id indices."""
    if getattr(bacc, "_ant_act_tables_patched", False):
        return
    real = bacc.get_activation_tables

    def patched(module_arch):
        tabs = real(module_arch)
        keep = {"natural_log_exp_and_others"}
        return {
            name: (fns if name in keep else set()) for name, fns in tabs.items()
        }

    bacc.get_activation_tables = patched
    bacc._ant_act_tables_patched = True


def _get_program(with_b=False):
    key = with_b
    if key in _CACHE:
        return _CACHE[key]
    _patch_act_tables()
    nc = bacc.Bacc("TRN2", target_bir_lowering=False, debug=False)
    aps = {}
    aps["u1"] = nc.dram_tensor(
        "u1", [NBLK, 128, 4 * N * BLK], f16, kind="ExternalInput"
    ).ap()
    aps["u2"] = nc.dram_tensor(
        "u2", [NBLK, 128, 4 * N * J * P4], f16, kind="ExternalInput"
    ).ap()
    wts = _build_weight_arrays()
    aps["wv"] = nc.dram_tensor("wv", [128, 128], f16, kind="ExternalInput").ap()
    aps["wc_band"] = nc.dram_tensor(
        "wc_band", [128, 248], f16, kind="ExternalInput"
    ).ap()
    aps["ws_band"] = nc.dram_tensor(
        "ws_band", [128, 134], f16, kind="ExternalInput"
    ).ap()
    aps["wn"] = nc.dram_tensor("wn", [128, 128], f16, kind="ExternalInput").ap()
    if with_b:
        aps["bt"] = nc.dram_tensor(
            "bt", [128, 4 * N * P4], f32, kind="ExternalInput"
        ).ap()
    aps["out"] = nc.dram_tensor(
        "out", [NBLK, 64, 4 * 8 * 4 * P4], f16, kind="ExternalOutput"
    ).ap()

    with tile.TileContext(nc) as tc:
        with ExitStack() as ctx:
            _emit(ctx, tc, aps, with_b)
    nc.compile()

    _CACHE[key] = (nc, wts)
    return _CACHE[key]


def kernel(u: np.ndarray, b: np.ndarray) -> np.ndarray:
    u = np.asarray(u, dtype=np.float32)
    b = np.asarray(b, dtype=np.float32)
    with_b = bool(np.any(b))
    nc, wts = _get_program(with_b=with_b)

    base = {
        "wv": wts["wv"].astype(np.float16),
        "wc_band": wts["wc_band"].astype(np.float16),
        "ws_band": wts["ws_band"].astype(np.float16),
        "wn": wts["wn"].astype(np.float16),
    }
    if with_b:
        base["bt"] = _b_tile_array(b)
    in_maps = []
    for c in range(NCORES):
        bb = c // 2
        h0 = 16 * (c % 2)
        shard = u[bb, :, :, :, h0 : h0 + 16, :].reshape(I, N, J, PIX)
        m = dict(base)
        m["u1"] = encode_u1(shard)
        m["u2"] = encode_u2(shard)
        in_maps.append(m)

    res = run_bass_kernel_spmd(nc, in_maps, core_ids=list(range(NCORES)))
    out = np.zeros((B, N, J, H, W), np.float32)
    for c in range(NCORES):
        bb = c // 2
        h0 = 16 * (c % 2)
        out[bb, :, :, h0 : h0 + 16, :] = decode_out(res.results[c]["out"]).reshape(
            N, J, 16, W
        )
    return out


# revision 35
# speedup vs baseline: 1.3557x; 1.0562x over previous
"""Trainium2 Bass kernel for capsule attention-routing.

Reference computation (per pixel; 4096 independent problems of shape
[I=32 in-caps, N=32 out-caps, J=16 caps-dim]):
    v[n,j]   = sum_i u[i,n,j]
    cp[i,n]  = sum_j u[i,n,j] * v[n,j] / 4
    c[i,n]   = softmax_n(cp)[i,n] + b[i,n]
    s[n,j]   = sum_i u[i,n,j] * c[i,n]
    out[n,j] = (1 - exp(-|s|_j)) * s[n,j] / |s|_j

Sharding: data-parallel over (batch, h-half): 8 cores x 512 pixels.

Per-core strategy (dual layout, u streamed twice from HBM as fp16):
  L1 (j-major): partitions (j*8+il), free (ib, n, p64)  [il=i%8, i=ib*8+il]
     - v-pass: PE contracts il (+PSUM accum over ib), broadcast over rows
     - c-mult: DVE 2x-mode fp16 w = u1 * v
     - c-red : PE contracts j via banded 0.25-delta weights ->
               cp[(g*8+il) parts, (ib,n,p4)]  [g = pixel>>2]
  softmax over n on the small cp tile (Act exp f32, Pool z-reduce,
  DVE reciprocal+mult -> c_sb fp16)
  L2 (i-major): partitions (g*8+il), free (ib, n, j, p4)
     - s-mult: DVE 2x-mode m2 = u2 * broadcast_j(c_sb)  (no PE broadcast,
               no PSUM->SBUF copies: c broadcasts via a stride-0 free dim)
     - s-red : PE contracts il within g strips (+accum over ib) ->
               s[(g,x8-replicated) parts, (n8,j,p4)] in 4 nq PSUM banks
  squash: Act square, Pool j-reduce, r = exp(.5*ln(n2)) / rn = exp(-.5*ln n2)
  (single act table: no LoadActFuncSet flips), Pool final multiply.
Softmax runs without max-subtraction (|cp| <~ 45 is safe in fp32 exp).
EPS=1e-20 is negligible: 1-1/(exp(r)+eps) == 1-exp(-r), 1/(r+eps) == 1/r.
"""

import numpy as np
from contextlib import ExitStack

import concourse.bass as bass
import concourse.bacc as bacc
import concourse.tile as tile
import concourse.mybir as mybir
from concourse.bass_utils import run_bass_kernel_spmd

dt = mybir.dt
AF = mybir.ActivationFunctionType
OP = mybir.AluOpType

B, I, N, J, H, W = 4, 32, 32, 16, 32, 32
HW = H * W
NCORES = 8
PIX = B * HW // NCORES      # 512 pixels per core
BLK = 64                    # pixels per block
NBLK = PIX // BLK           # 8
NG = 16                     # pixel groups of 4 per block (g = pixel>>2)
P4 = 4
SCALE = 0.25                # 1/sqrt(16)

f32, bf16, f16 = dt.float32, dt.bfloat16, dt.float16


def _build_weight_arrays():
    il_of = np.arange(128) % 8          # L1 partition -> il is p%8? no: p=(j,il)
    # L1 partitions: p = j*8 + il  -> j = p//8, il = p%8
    j_of = np.arange(128) // 8
    il1 = np.arange(128) % 8

    # v-pass: out[(j2,il2)] = sum_il u[(j,il)] for j==j2 (broadcast over il2)
    wv = np.zeros((128, 128), np.float32)
    for p_in in range(128):
        for p_out in range(128):
            if j_of[p_in] == j_of[p_out]:
                wv[p_in, p_out] = 1.0

    # c-red band: window at offset off(g)=2*(120 - g*8) bytes gives the
    # [128,128] weight mapping (j,il) -> out partition (g*8+il), scaled 0.25.
    # band[(j,il), c] = 0.25 iff c == 120 + il
    wc_band = np.zeros((128, 248), np.float32)
    for p_in in range(128):
        wc_band[p_in, 120 + il1[p_in]] = SCALE

    # s-red band: window at offset off(jq)=2*(6 - jq*2) bytes maps L2
    # partitions (g,il) -> out partition (g*8 + jq*2 + r), r=0,1 replicas.
    # band[(g,il), c] = 1 iff c in (g*8+6, g*8+7)
    ws_band = np.zeros((128, 134), np.float32)
    g_of = np.arange(128) // 8
    for p_in in range(128):
        ws_band[p_in, g_of[p_in] * 8 + 6] = 1.0
        ws_band[p_in, g_of[p_in] * 8 + 7] = 1.0

    # n2: contract the 8 rows of each g strip (each real value appears
    # twice via the r2 replicas -> 0.5)
    wn = np.zeros((128, 128), np.float32)
    for p_in in range(128):
        for p_out in range(128):
            if p_in // 8 == p_out // 8:
                wn[p_in, p_out] = 0.5

    return {"wv": wv, "wc_band": wc_band, "ws_band": ws_band, "wn": wn}


def _b_tile_array(b_np):
    # bt[(g*8+il), (ib, n, p4)] = b[ib*8+il, n]
    bt = np.zeros((128, 4 * N * P4), np.float32)
    bsl = np.asarray(b_np).reshape(I, N)
    for g in range(NG):
        for il in range(8):
            row = g * 8 + il
            for ib in range(4):
                for n in range(N):
                    bt[row, (ib * N + n) * P4 : (ib * N + n + 1) * P4] = bsl[
                        ib * 8 + il, n
                    ]
    return bt


def _emit(ctx: ExitStack, tc: tile.TileContext, aps: dict, with_b: bool):
    nc = tc.nc
    u1_d, u2_d, o_d = aps["u1"], aps["u2"], aps["out"]

    # constants
    pconst = ctx.enter_context(tc.tile_pool(name="const", bufs=1))
    wv_t = pconst.tile([128, 128], f16, tag="wv")
    wcb_t = pconst.tile([128, 248], f16, tag="wcb")
    ws_t = pconst.tile([128, 134], f16, tag="ws")
    wn_t = pconst.tile([128, 128], f16, tag="wn")
    nc.sync.dma_start(wv_t[:], aps["wv"])
    nc.sync.dma_start(wcb_t[:], aps["wc_band"])
    nc.sync.dma_start(ws_t[:], aps["ws_band"])
    nc.sync.dma_start(wn_t[:], aps["wn"])
    bt_t = None
    if with_b:
        bt_t = pconst.tile([128, 4 * N * P4], f32, tag="bt")
        nc.sync.dma_start(bt_t[:], aps["bt"])

    # pools
    pu1 = ctx.enter_context(tc.tile_pool(name="u1", bufs=3))
    pu2 = ctx.enter_context(tc.tile_pool(name="u2", bufs=2))
    pw1 = ctx.enter_context(tc.tile_pool(name="w1", bufs=2))
    pm2 = ctx.enter_context(tc.tile_pool(name="m2", bufs=3))
    pvsb = ctx.enter_context(tc.tile_pool(name="vsb", bufs=3))
    pce = ctx.enter_context(tc.tile_pool(name="ce", bufs=2))
    pcsb = ctx.enter_context(tc.tile_pool(name="csb", bufs=2))
    psq = ctx.enter_context(tc.tile_pool(name="sq", bufs=2))
    pout = ctx.enter_context(tc.tile_pool(name="out", bufs=2))

    pcsb2 = ctx.enter_context(tc.tile_pool(name="csb2", bufs=3))

    pvps = ctx.enter_context(tc.tile_pool(name="vps", bufs=2, space="PSUM"))
    pcps = ctx.enter_context(tc.tile_pool(name="cps", bufs=2, space="PSUM"))
    psps = ctx.enter_context(tc.tile_pool(name="sps", bufs=3, space="PSUM"))
    pnps = ctx.enter_context(tc.tile_pool(name="nps", bufs=1, space="PSUM"))

    def load(blk):
        u1 = pu1.tile([128, 4 * N * BLK], f16, tag="u1")   # [(j,il),(ib,n,p64)]
        nc.sync.dma_start(u1[:], u1_d[blk])
        u2 = pu2.tile([128, 4 * N * J * P4], f16, tag="u2")  # [(g,il),(ib,n,j,p4)]
        nc.sync.dma_start(u2[:], u2_d[blk])
        return u1, u2

    loads1 = {}
    loads2 = {}

    def load1(blk):
        u1 = pu1.tile([128, 4 * N * BLK], f16, tag="u1")   # [(j,il),(ib,n,p64)]
        nc.sync.dma_start(u1[:], u1_d[blk])
        loads1[blk] = u1

    def load2(blk):
        u2 = pu2.tile([128, 4 * N * J * P4], f16, tag="u2")  # [(g,il),(ib,n,j,p4)]
        nc.sync.dma_start(u2[:], u2_d[blk])
        loads2[blk] = u2

    def s1(blk):
        """v-pass (PE) -> v-copies (Pool); prefetch next u1."""
        if blk + 1 < NBLK:
            load1(blk + 1)
        u1 = loads1.pop(blk)

        u1_v = u1[:].rearrange("P (ib n p) -> P ib n p", ib=4, p=BLK)

        # ---- v-pass (PE): v[(j,il-bcast),(n,p64)] = sum_i u1 ----
        v_sb = pvsb.tile([128, N * BLK], f16, tag="vsb")
        v_sb_v = v_sb[:].rearrange("P (n p) -> P n p", p=BLK)
        for st in range(4):
            v_ps = pvps.tile([128, 512], f32, tag="vps")
            v_ps_v = v_ps[:].rearrange("P (n p) -> P n p", p=16)
            for ib in range(4):
                nc.tensor.matmul(
                    v_ps_v,
                    wv_t[:],
                    u1_v[:, ib, :, st * 16 : (st + 1) * 16],
                    start=(ib == 0),
                    stop=(ib == 3),
                )
            nc.scalar.copy(v_sb_v[:, :, st * 16 : (st + 1) * 16], v_ps_v)
        return u1, v_sb

    def s2(blk, u1, v_sb):
        """c-mult (DVE) -> c-red (PE) -> softmax; prefetch u2."""
        load2(blk)
        u1_v = u1[:].rearrange("P (ib n p) -> P ib n p", ib=4, p=BLK)
        v_sb_v = v_sb[:].rearrange("P (n p) -> P n p", p=BLK)

        # ---- c-mult (DVE 2x): w1 = u1 * v ----
        w1 = pw1.tile([128, 4 * N * BLK], f16, tag="w1")
        w1_v = w1[:].rearrange("P (ib n p) -> P ib n p", ib=4, p=BLK)
        for ib in range(4):
            nc.vector.tensor_tensor(
                w1_v[:, ib], u1_v[:, ib], v_sb_v, op=OP.mult
            )

        # ---- c-red (PE): cp[(g,il), (ib,n,p4)] = 0.25*sum_j w1 ----
        cp = pcps.tile([128, 4 * N * P4], f32, tag="cp")
        cp_v = cp[:].rearrange("P (ib n p) -> P ib n p", ib=4, p=P4)
        for g in range(NG):
            off = 120 - g * 8
            nc.tensor.matmul(
                cp_v,
                wcb_t[:, off : off + 128],
                w1_v[:, :, :, g * P4 : (g + 1) * P4],
                start=(g == 0),
                stop=(g == NG - 1),
                skip_group_check=True,
            )

        # ---- softmax over n (no max-subtraction) ----
        c_e = pce.tile([128, 4 * N * P4], f32, tag="ce")
        nc.scalar.activation(c_e[:], cp[:], AF.Exp)
        c_e_v = c_e[:].rearrange("P (ib n p) -> P ib n p", ib=4, p=P4)
        z = pcsb.tile([128, 4 * P4], f32, tag="z")
        nc.vector.tensor_reduce(
            z[:].rearrange("P (ib p) -> P ib p", ib=4),
            c_e[:].rearrange("P (ib n p) -> P ib p n", ib=4, p=P4),
            axis=mybir.AxisListType.X,
            op=OP.add,
        )
        rz = pcsb.tile([128, 4 * P4], f32, tag="rz")
        nc.vector.reciprocal(rz[:], z[:])
        rz_b = (
            rz[:]
            .rearrange("P (ib o p) -> P ib o p", ib=4, o=1)
            .broadcast_to([128, 4, N, P4])
        )
        c_sb = pcsb2.tile([128, 4 * N * P4], f16, tag="csb")
        c_sb_v = c_sb[:].rearrange("P (ib n p) -> P ib n p", ib=4, p=P4)
        if with_b:
            c_f = pcsb.tile([128, 4 * N * P4], f32, tag="cf")
            nc.gpsimd.tensor_tensor(
                c_f[:].rearrange("P (ib n p) -> P ib n p", ib=4, p=P4),
                c_e_v,
                rz_b,
                op=OP.mult,
            )
            nc.gpsimd.tensor_tensor(c_sb[:], c_f[:], bt_t[:], op=OP.add)
        else:
            nc.gpsimd.tensor_tensor(c_sb_v, c_e_v, rz_b, op=OP.mult)
        return c_sb

    def s3(blk, c_sb):
        """m2 -> s-red -> squash -> store."""
        u2 = loads2.pop(blk)
        c_sb_v = c_sb[:].rearrange("P (ib n p) -> P ib n p", ib=4, p=P4)
        # s_all[(g, jq, r2) parts, (nq, m8, jl4, p4)]; j = jq*4 + jl
        u2_v = u2[:].rearrange("P (ib n j p) -> P ib n j p", ib=4, n=N, p=P4)
        s_all = psps.tile([128, 4 * 8 * 4 * P4], f32, tag="sall")
        s_all_v = s_all[:].rearrange(
            "P (q m jl p) -> P q m jl p", q=4, m=8, p=P4
        )
        s_all_v2 = s_all[:].rearrange("P (n jl p) -> P n jl p", n=N, p=P4)
        for ib in range(4):
            m2 = pm2.tile([128, N * J * P4], f16, tag="m2")
            m2_v = m2[:].rearrange("P (n j p) -> P n j p", n=N, p=P4)
            cb = (
                c_sb_v[:, ib]
                .rearrange("P n (o p) -> P n o p", o=1)
                .broadcast_to([128, N, J, P4])
            )
            nc.vector.tensor_tensor(m2_v, u2_v[:, ib], cb, op=OP.mult)
            for jq in range(4):
                off = 6 - jq * 2
                nc.tensor.matmul(
                    s_all_v2,
                    ws_t[:, off : off + 128],
                    m2_v[:, :, jq * 4 : (jq + 1) * 4],
                    start=(ib == 0 and jq == 0),
                    stop=(ib == 3 and jq == 3),
                    skip_group_check=True,
                )

        # ---- squash ----
        # ssq = s^2 (bf16 keeps fp32 range; fp16 would flush subnormals)
        ssq = psq.tile([128, 4 * 8 * 4 * P4], bf16, tag="ssq")
        nc.scalar.activation(ssq[:], s_all[:], AF.Square)
        ssq_v = ssq[:].rearrange("P (q m jl p) -> P q m jl p", q=4, m=8, p=P4)
        t1 = psq.tile([128, 4 * 8 * 2 * P4], bf16, tag="t1")
        t1_v = t1[:].rearrange("P (q m jl p) -> P q m jl p", q=4, m=8, p=P4)
        nc.gpsimd.tensor_tensor(
            t1_v, ssq_v[:, :, :, 0:2], ssq_v[:, :, :, 2:4], op=OP.add
        )
        ssq_l = psq.tile([128, 4 * 8 * P4], bf16, tag="ssql")
        nc.gpsimd.tensor_tensor(
            ssq_l[:].rearrange("P (q m p) -> P q m p", q=4, p=P4),
            t1_v[:, :, :, 0],
            t1_v[:, :, :, 1],
            op=OP.add,
        )
        # n2[(g,x8), (nq,m,p4)] = sum_j s^2 via PE partition contraction
        n2 = pnps.tile([128, 4 * 8 * P4], f32, tag="n2")
        nc.tensor.matmul(n2[:], wn_t[:], ssq_l[:], start=True, stop=True)
        # clamp away 0 so ln is finite; out is ~0 there anyway
        n2c = psq.tile([128, 4 * 8 * P4], f32, tag="n2c")
        nc.gpsimd.tensor_scalar(n2c[:], n2[:], 1e-30, None, op0=OP.max)
        lnn = psq.tile([128, 4 * 8 * P4], f32, tag="lnn")
        nc.scalar.activation(lnn[:], n2c[:], AF.Ln)
        # r = exp(.5 ln n2) = |s|; rn = exp(-.5 ln n2) = 1/|s|
        r_t = psq.tile([128, 4 * 8 * P4], f32, tag="r")
        nc.scalar.activation(r_t[:], lnn[:], AF.Exp, scale=0.5)
        rn_t = psq.tile([128, 4 * 8 * P4], f32, tag="rn")
        nc.scalar.activation(rn_t[:], lnn[:], AF.Exp, scale=-0.5)
        en_t = psq.tile([128, 4 * 8 * P4], f32, tag="en")
        nc.scalar.activation(en_t[:], r_t[:], AF.Exp, scale=-1.0)
        g_t = psq.tile([128, 4 * 8 * P4], f32, tag="g")
        nc.gpsimd.scalar_tensor_tensor(
            g_t[:], en_t[:], 1.0, rn_t[:], op0=OP.subtract, op1=OP.mult
        )  # g = (en - 1) / r
        g_b = (
            g_t[:]
            .rearrange("P (q m o p) -> P q m o p", q=4, m=8, o=1)
            .broadcast_to([128, 4, 8, 4, P4])
        )

        outt = pout.tile([128, 4 * 8 * 4 * P4], f16, tag="outt")
        nc.gpsimd.scalar_tensor_tensor(
            outt[:].rearrange("P (q m jl p) -> P q m jl p", q=4, m=8, p=P4),
            s_all_v,
            -1.0,
            g_b,
            op0=OP.mult,
            op1=OP.mult,
        )  # out = (-s) * g = s * (1-en)/r

        # only the r=0 replicas carry data: 64 partitions, stride 2
        nc.sync.dma_start(o_d[blk], outt[::2, :])

    # 3-stage software pipeline: emit s1(k), s2(k-1), s3(k-2) per iteration
    # so each in-order engine queue sees work in expected-ready order (the
    # PE queue in particular becomes v(k), c-red(k-1), s-red(k-2), each of
    # whose inputs is already in flight — PE stays continuously busy and at
    # full p-state).
    p1, p2 = {}, {}
    load1(0)
    for it in range(NBLK + 2):
        if it < NBLK:
            p1[it] = s1(it)
        if 1 <= it <= NBLK:
            p2[it - 1] = s2(it - 1, *p1.pop(it - 1))
        if it >= 2:
            s3(it - 2, p2.pop(it - 2))


def round_f16(x):
    return x.astype(np.float16)


def encode_u1(shard):
    """[I, N, J, pix] -> [blk, (j,il)=128, (ib,n,p64)] fp16."""
    a = shard.reshape(4, 8, N, J, NBLK, BLK)          # ib, il, n, j, blk, p
    # -> blk, j, il, ib, n, p
    return np.ascontiguousarray(
        a.transpose(4, 3, 1, 0, 2, 5)
    ).astype(np.float16)


def encode_u2(shard):
    """[I, N, J, pix] -> [blk, (g,il)=128, (ib,n,j,p4)] fp16."""
    a = shard.reshape(4, 8, N, J, NBLK, NG, P4)       # ib, il, n, j, blk, g, p4
    # -> blk, g, il, ib, n, j, p4
    return np.ascontiguousarray(
        a.transpose(4, 5, 1, 0, 2, 3, 6)
    ).astype(np.float16)


def decode_out(arr):
    """[blk, 64=(g,jq), (nq,m8,jl4,p4)] fp16 -> [N, J, pix] f32.

    n = nq*8+m; j = jq*4+jl; pixel = blk*64 + g*4 + p
    """
    a = arr.astype(np.float32).reshape(NBLK, NG, 4, 4, 8, 4, P4)
    # dims: blk, g, jq, nq, m, jl, p -> (nq,m), (jq,jl), (blk,g,p)
    return np.ascontiguousarray(a.transpose(3, 4, 2, 5, 0, 1, 6)).reshape(
        N, J, PIX
    )


_CACHE = {}


def _patch_act_tables():
    """Keep only natural_log_exp_and_others (Copy/Exp/Ln/Square): every
    function this kernel uses lives in one table, so exactly ONE
    LoadActFuncSet is emitted. Other set entries are kept (emptied) to
    preserve act_func_set_id indices."""
    if getattr(bacc, "_ant_act_tables_patched", False):
        return
    real = bacc.get_activation_tables

    def patched(module_arch):
        tabs = real(module_arch)
        keep = {"natural_log_exp_and_others"}
        return {
            name: (fns if name in keep else set()) for name, fns in tabs.items()
        }

    bacc.get_activation_tables = patched
    bacc._ant_act_tables_patched = True


def _get_program(with_b=False):
    key = with_b
    if key in _CACHE:
        return _CACHE[key]
    _patch_act_tables()
    nc = bacc.Bacc("TRN2", target_bir_lowering=False, debug=False)
    aps = {}
    aps["u1"] = nc.dram_tensor(
        "u1", [NBLK, 128, 4 * N * BLK], f16, kind="ExternalInput"
    ).ap()
    aps["u2"] = nc.dram_tensor(
        "u2", [NBLK, 128, 4 * N * J * P4], f16, kind="ExternalInput"
    ).ap()
    wts = _build_weight_arrays()
    aps["wv"] = nc.dram_tensor("wv", [128, 128], f16, kind="ExternalInput").ap()
    aps["wc_band"] = nc.dram_tensor(
        "wc_band", [128, 248], f16, kind="ExternalInput"
    ).ap()
    aps["ws_band"] = nc.dram_tensor(
        "ws_band", [128, 134], f16, kind="ExternalInput"
    ).ap()
    aps["wn"] = nc.dram_tensor("wn", [128, 128], f16, kind="ExternalInput").ap()
    if with_b:
        aps["bt"] = nc.dram_tensor(
            "bt", [128, 4 * N * P4], f32, kind="ExternalInput"
        ).ap()
    aps["out"] = nc.dram_tensor(
        "out", [NBLK, 64, 4 * 8 * 4 * P4], f16, kind="ExternalOutput"
    ).ap()

    with tile.TileContext(nc) as tc:
        with ExitStack() as ctx:
            _emit(ctx, tc, aps, with_b)
    nc.compile()

    _CACHE[key] = (nc, wts)
    return _CACHE[key]


def kernel(u: np.ndarray, b: np.ndarray) -> np.ndarray:
    u = np.asarray(u, dtype=np.float32)
    b = np.asarray(b, dtype=np.float32)
    with_b = bool(np.any(b))
    nc, wts = _get_program(with_b=with_b)

    base = {
        "wv": wts["wv"].astype(np.float16),
        "wc_band": wts["wc_band"].astype(np.float16),
        "ws_band": wts["ws_band"].astype(np.float16),
        "wn": wts["wn"].astype(np.float16),
    }
    if with_b:
        base["bt"] = _b_tile_array(b)
    in_maps = []
    for c in range(NCORES):
        bb = c // 2
        h0 = 16 * (c % 2)
        shard = u[bb, :, :, :, h0 : h0 + 16, :].reshape(I, N, J, PIX)
        m = dict(base)
        m["u1"] = encode_u1(shard)
        m["u2"] = encode_u2(shard)
        in_maps.append(m)

    res = run_bass_kernel_spmd(nc, in_maps, core_ids=list(range(NCORES)))
    out = np.zeros((B, N, J, H, W), np.float32)
    for c in range(NCORES):
        bb = c // 2
        h0 = 16 * (c % 2)
        out[bb, :, :, h0 : h0 + 16, :] = decode_out(res.results[c]["out"]).reshape(
            N, J, 16, W
        )
    return out
